# revision 20
# baseline (speedup 1.0000x reference)
"""LoRA basis-bank kernel for 8 TRN2 NeuronCores.

Math (per batch b):
    A_mixed  = sum_k alpha[b,k] * A_bank[k]        # [R, DIN]
    B_mixedT = sum_k alpha[b,k] * B_bank[k].T      # [R, DOUT]
    z        = h[b] @ A_mixed.T                    # [S, R]
    delta[b] = z @ B_mixedT                        # [S, DOUT]

Sharding: data-parallel over batch, 1 batch per core; banks replicated.

Host-side layout prep (no arithmetic): h shard is uploaded transposed
and wave-tiled ([wave, group, partition, 4*256] bf16) so every load is
a clean 2D DMA with contiguous rows; mix/A_bank/B_bank.T are
concatenated+folded into one [128, 2*(R+DIN+DOUT)] bf16 "banks"
tensor; alpha is expanded into a [K*R, R] block-diagonal matrix.

Device dataflow per core — wave-pipelined streaming schedule that keeps
the DMA engines busy end to end (memory roofline):
  - S is processed in 8 waves of 256 rows.  Loads (sync/HWDGE ring):
    mix+A banks, wave-0 hT, B bank, wave-1..7 hT.  Stores go out on the
    gpsimd/SWDGE ring so they interleave with the queued loads at the
    SDMA level instead of waiting behind them.
  - A_mixT chunks land in ONE rotating PSUM buffer, one cast.
  - B_mixedT replicated at partition strips 0-15 / 32-47 (bmix2),
    drains split across vector/scalar.
  - Per wave: mm1 c-outer consumes each arriving hT group tile into a
    bank-padded PSUM accumulator zT [16, 256]; cast to both strips;
    mm2 runs 2-way row-tiled (tile_position (0,0)/(32,0)) with each
    concurrent matmul pair writing one 2-bank [128, 1024] PSUM tile
    drained by a single wide copy (vector/scalar alternate); one store
    per 128-row s-tile.
  - Wave k+1's loads stream underneath wave k's mm2+stores, so the
    mm1->cast->mm2->copy->store chain is exposed only on the last wave.
  - delta written bf16, upcast to fp32 on host.
"""

import ml_dtypes
import numpy as np

import concourse.bacc as bacc
import concourse.bass as bass
import concourse.mybir as mybir
import concourse.tile as tile
from concourse.bass_utils import run_bass_kernel_spmd

B, S, K, R, DIN, DOUT = 8, 2048, 16, 16, 2048, 2048
KR = K * R  # 256
BANKW = R + DIN + DOUT  # 4112
MA = R + DIN  # 2064: mix + A columns
W = 256  # wave width (s-columns per wave)
NW = S // W  # 8 waves
F32 = mybir.dt.float32
BF16 = mybir.dt.bfloat16

_cache = {}


def _build_nc():
    nc = bacc.Bacc("TRN2", target_bir_lowering=False)

    # h uploaded wave-tiled: [wave, group, partition, 4*W]
    ht_d = nc.dram_tensor("hbT", [NW, 4, 128, 4 * W], BF16,
                          kind="ExternalInput")
    bank_d = nc.dram_tensor("banks", [128, 2 * BANKW], BF16,
                            kind="ExternalInput")
    out_d = nc.dram_tensor("delta", [S, DOUT], BF16, kind="ExternalOutput")

    NCH = DIN // 128  # 16 chunks along DIN
    NG = 4            # hT group tiles per wave (4 chunks each)
    with tile.TileContext(nc) as tc:
        with (
            tc.tile_pool(name="const", bufs=1) as constp,
            tc.tile_pool(name="hT", bufs=4) as hTp,
            tc.tile_pool(name="zts", bufs=1) as ztsp,
            tc.tile_pool(name="dout", bufs=4) as dp,
            tc.tile_pool(name="psz", bufs=1, space="PSUM") as pszp,
            tc.tile_pool(name="psd", bufs=3, space="PSUM") as psdp,
        ):
            # ---- loads on the sync ring, in issue order ----
            bank = constp.tile([128, 2 * BANKW], BF16, tag="bank")
            # mix + A first (amix gates wave-0's mm1)
            nc.sync.dma_start(bank[:, 0:MA], bank_d[:, 0:MA])
            nc.sync.dma_start(bank[:, BANKW:BANKW + MA],
                              bank_d[:, BANKW:BANKW + MA])

            def load_wave(sc):
                tiles = []
                for gi in range(NG):
                    hT = hTp.tile([128, NG * W], BF16, tag=f"g{gi}",
                                  name=f"hT_{sc}_{gi}")
                    nc.sync.dma_start(hT[:], ht_d[sc, gi])
                    tiles.append(hT)
                return tiles

            waves = [load_wave(0)]
            # B bank lands right after wave-0 so bmix2 is ready for mm2
            nc.sync.dma_start(bank[:, MA:BANKW], bank_d[:, MA:BANKW])
            nc.sync.dma_start(bank[:, BANKW + MA:2 * BANKW],
                              bank_d[:, BANKW + MA:2 * BANKW])
            waves += [load_wave(sc) for sc in range(1, NW)]

            m_sb = [bank[:, 0:R], bank[:, BANKW:BANKW + R]]
            a_sb = [bank[:, R:MA], bank[:, BANKW + R:BANKW + MA]]
            b_sb = [bank[:, MA:BANKW], bank[:, BANKW + MA:2 * BANKW]]

            # ---- A_mixT: all 16 chunks into one PSUM buffer ----
            amix_ps = psdp.tile([128, 1024], F32, tag="dps", name="amix_ps")
            for c in range(NCH):
                csl = slice(c * 128, (c + 1) * 128)
                osl = slice(c * R, (c + 1) * R)
                nc.tensor.matmul(amix_ps[:, osl], a_sb[0][:, csl], m_sb[0],
                                 start=True, stop=False)
                nc.tensor.matmul(amix_ps[:, osl], a_sb[1][:, csl], m_sb[1],
                                 start=False, stop=True)
            amixT = constp.tile([128, NCH * R], BF16, tag="amixT")
            nc.vector.tensor_copy(amixT[:], amix_ps[:, 0:NCH * R])

            def mm1(sc):
                # bank-padded accumulator: adjacent waves must not share
                # a PSUM bank (PE-write vs cast-read would collide)
                zt_ps = pszp.tile([R, 512], F32, tag=f"zt{sc % 2}",
                                  name=f"zt_ps{sc}")
                for gi in range(NG):
                    for g in range(NG):
                        c = gi * NG + g
                        nc.tensor.matmul(
                            zt_ps[:, 0:W], amixT[:, c * R:(c + 1) * R],
                            waves[sc][gi][:, g * W:(g + 1) * W],
                            start=(c == 0), stop=(c == NCH - 1))
                return zt_ps

            def mm2_wave(sc, zts2):
                for t in range(W // 128):
                    st = sc * (W // 128) + t
                    dsb = dp.tile([128, DOUT], BF16, tag="d",
                                  name=f"dsb{st}")
                    for ocp in (0, 2):
                        dps2 = psdp.tile([128, 1024], F32, tag="dps",
                                         name=f"dps{st}_{ocp}")
                        tsl = slice(t * 128, (t + 1) * 128)
                        nc.tensor.matmul(
                            dps2[:, 0:512], zts2[0:R, tsl],
                            bmix2[0:R, ocp * 512:(ocp + 1) * 512],
                            tile_position=(0, 0))
                        nc.tensor.matmul(
                            dps2[:, 512:1024], zts2[32:32 + R, tsl],
                            bmix2[32:32 + R,
                                  (ocp + 1) * 512:(ocp + 2) * 512],
                            tile_position=(32, 0))
                        osl = slice(ocp * 512, (ocp + 2) * 512)
                        if ocp == 0:
                            nc.vector.tensor_copy(dsb[:, osl], dps2[:])
                        else:
                            nc.scalar.copy(dsb[:, osl], dps2[:])
                    nc.gpsimd.dma_start(
                        out_d[st * 128:(st + 1) * 128, :], dsb[:])

            # wave-0 mm1 next on the PE (amixT + wave-0 data gate it)
            zt_cur = mm1(0)

            # ---- B_mixedT replicated at strips 0-15 / 32-47 ----
            bmix2 = constp.tile([48, DOUT], BF16, tag="bmix2")
            for c4 in range(DOUT // 512):
                sl = slice(c4 * 512, (c4 + 1) * 512)
                pmix = psdp.tile([R, 512], F32, tag="dps", name=f"pmix{c4}")
                nc.tensor.matmul(pmix[:], m_sb[0], b_sb[0][:, sl],
                                 start=True, stop=False)
                nc.tensor.matmul(pmix[:], m_sb[1], b_sb[1][:, sl],
                                 start=False, stop=True)
                if c4 % 2 == 0:
                    nc.vector.tensor_copy(bmix2[0:R, sl], pmix[:])
                    nc.vector.tensor_copy(bmix2[32:32 + R, sl], pmix[:])
                else:
                    nc.scalar.copy(bmix2[0:R, sl], pmix[:])
                    nc.scalar.copy(bmix2[32:32 + R, sl], pmix[:])

            for sc in range(NW):
                zts2 = ztsp.tile([48, W], BF16, tag=f"z{sc % 2}",
                                 name=f"zts2_{sc}")
                nc.vector.tensor_copy(zts2[0:R, :], zt_cur[:, 0:W])
                nc.scalar.copy(zts2[32:32 + R, :], zt_cur[:, 0:W])
                mm2_wave(sc, zts2)
                if sc + 1 < NW:
                    zt_cur = mm1(sc + 1)

    nc.compile()
    return nc


def _in_maps(h, alpha, A_bank, B_bank):
    a_flat = A_bank.reshape(KR, DIN).astype(np.float32)
    bt_flat = B_bank.transpose(0, 2, 1).reshape(KR, DOUT).astype(np.float32)
    eye = np.eye(R, dtype=np.float32)
    maps = []
    for b in range(B):
        mix = np.kron(alpha[b].astype(np.float32).reshape(K, 1), eye)
        banks = np.concatenate([mix, a_flat, bt_flat], axis=1)
        banks = banks.reshape(2, 128, BANKW).transpose(1, 0, 2).reshape(
            128, 2 * BANKW)
        hT = np.asarray(h[b]).T  # [DIN, S]
        # wave-tile: [sc, gi, p, g*W+s] = hT[(gi*4+g)*128 + p, sc*W+s]
        hTw = hT.reshape(4, 4, 128, NW, W).transpose(3, 0, 2, 1, 4)
        hTw = hTw.reshape(NW, 4, 128, 4 * W)
        maps.append({
            "hbT": np.ascontiguousarray(hTw).astype(ml_dtypes.bfloat16),
            "banks": np.ascontiguousarray(banks.astype(ml_dtypes.bfloat16)),
        })
    return maps


def _run(inputs, trace=False):
    if "nc" not in _cache:
        _cache["nc"] = _build_nc()
    nc = _cache["nc"]
    maps = _in_maps(inputs["h"], inputs["alpha"], inputs["A_bank"],
                    inputs["B_bank"])
    res = run_bass_kernel_spmd(nc, maps, core_ids=list(range(B)), trace=trace)
    out = np.stack([res.results[b]["delta"] for b in range(B)], axis=0)
    return out.astype(np.float32), res


def kernel(**inputs):
    out, _ = _run(inputs, trace=False)
    return out


# revision 21
# speedup vs baseline: 1.0259x; 1.0259x over previous
"""LoRA basis-bank kernel for 8 TRN2 NeuronCores.

Math (per batch b):
    A_mixed  = sum_k alpha[b,k] * A_bank[k]        # [R, DIN]
    B_mixedT = sum_k alpha[b,k] * B_bank[k].T      # [R, DOUT]
    z        = h[b] @ A_mixed.T                    # [S, R]
    delta[b] = z @ B_mixedT                        # [S, DOUT]

Sharding: data-parallel over batch, 1 batch per core; banks replicated.

Host-side layout prep (no arithmetic): h shard is uploaded transposed
and wave-tiled ([wave, group, partition, 4*512] bf16) so every load is
a clean 2D DMA with 4KB contiguous rows; mix/A_bank/B_bank.T are
concatenated+folded into one [128, 2*(R+DIN+DOUT)] bf16 "banks"
tensor; alpha is expanded into a [K*R, R] block-diagonal matrix.

Device dataflow per core — wave-pipelined streaming schedule that keeps
the DMA engines busy end to end (memory roofline):
  - S is processed in 4 waves of 512 rows.  Loads go on the sync/SP
    HWDGE ring; stores on the scalar/ACT HWDGE ring (the two HWDGE
    rings round-robin fairly at the SDMA level; SWDGE stores would be
    starved behind queued HWDGE loads).
  - A_mixT chunks land in ONE rotating PSUM buffer, one cast.
  - B_mixedT replicated at partition strips 0-15 / 32-47 (bmix2).
  - Per wave: mm1 c-outer consumes each arriving hT group tile into a
    1-bank PSUM accumulator zT [16, 512]; cast to both strips; mm2
    runs 2-way row-tiled (tile_position (0,0)/(32,0)) with each
    concurrent matmul pair writing one 2-bank [128, 1024] PSUM tile
    drained by a single wide copy (vector/scalar alternate) into a
    wave-stacked [128, 4, 2048] output tile; ONE 2MB store per wave
    (last wave stores its 4 s-tiles separately to shorten the tail).
  - Wave k+1's loads stream underneath wave k's mm2+stores, so the
    mm1->cast->mm2->copy->store chain is exposed only on the last wave.
  - delta written bf16, upcast to fp32 on host.
"""

import ml_dtypes
import numpy as np

import concourse.bacc as bacc
import concourse.bass as bass
import concourse.mybir as mybir
import concourse.tile as tile
from concourse.bass_utils import run_bass_kernel_spmd

B, S, K, R, DIN, DOUT = 8, 2048, 16, 16, 2048, 2048
KR = K * R  # 256
BANKW = R + DIN + DOUT  # 4112
MA = R + DIN  # 2064: mix + A columns
W = 512  # wave width (s-columns per wave)
NW = S // W  # 4 waves
F32 = mybir.dt.float32
BF16 = mybir.dt.bfloat16

_cache = {}


def _build_nc():
    nc = bacc.Bacc("TRN2", target_bir_lowering=False)

    # h uploaded wave-tiled: [wave, group, partition, 4*W]
    ht_d = nc.dram_tensor("hbT", [NW, 4, 128, 4 * W], BF16,
                          kind="ExternalInput")
    bank_d = nc.dram_tensor("banks", [128, 2 * BANKW], BF16,
                            kind="ExternalInput")
    out_d = nc.dram_tensor("delta", [S, DOUT], BF16, kind="ExternalOutput")

    NCH = DIN // 128  # 16 chunks along DIN
    NG = 4            # hT group tiles per wave (4 chunks each)
    NT = W // 128     # s-tiles per wave
    with tile.TileContext(nc) as tc:
        with (
            tc.tile_pool(name="const", bufs=1) as constp,
            tc.tile_pool(name="hT", bufs=3) as hTp,
            tc.tile_pool(name="zts", bufs=1) as ztsp,
            tc.tile_pool(name="dout", bufs=2) as dp,
            tc.tile_pool(name="psz", bufs=1, space="PSUM") as pszp,
            tc.tile_pool(name="psd", bufs=3, space="PSUM") as psdp,
        ):
            # ---- loads on the sync ring, in issue order ----
            bank = constp.tile([128, 2 * BANKW], BF16, tag="bank")
            # mix + A first (amix gates wave-0's mm1)
            nc.sync.dma_start(bank[:, 0:MA], bank_d[:, 0:MA])
            nc.sync.dma_start(bank[:, BANKW:BANKW + MA],
                              bank_d[:, BANKW:BANKW + MA])

            def load_wave(sc):
                tiles = []
                for gi in range(NG):
                    hT = hTp.tile([128, NG * W], BF16, tag=f"g{gi}",
                                  name=f"hT_{sc}_{gi}")
                    nc.sync.dma_start(hT[:], ht_d[sc, gi])
                    tiles.append(hT)
                return tiles

            waves = [load_wave(0)]
            # B bank lands right after wave-0 so bmix2 is ready for mm2
            nc.sync.dma_start(bank[:, MA:BANKW], bank_d[:, MA:BANKW])
            nc.sync.dma_start(bank[:, BANKW + MA:2 * BANKW],
                              bank_d[:, BANKW + MA:2 * BANKW])
            waves += [load_wave(sc) for sc in range(1, NW)]

            m_sb = [bank[:, 0:R], bank[:, BANKW:BANKW + R]]
            a_sb = [bank[:, R:MA], bank[:, BANKW + R:BANKW + MA]]
            b_sb = [bank[:, MA:BANKW], bank[:, BANKW + MA:2 * BANKW]]

            # ---- A_mixT: all 16 chunks into one PSUM buffer ----
            amix_ps = psdp.tile([128, 1024], F32, tag="dps", name="amix_ps")
            for c in range(NCH):
                csl = slice(c * 128, (c + 1) * 128)
                osl = slice(c * R, (c + 1) * R)
                nc.tensor.matmul(amix_ps[:, osl], a_sb[0][:, csl], m_sb[0],
                                 start=True, stop=False)
                nc.tensor.matmul(amix_ps[:, osl], a_sb[1][:, csl], m_sb[1],
                                 start=False, stop=True)
            amixT = constp.tile([128, NCH * R], BF16, tag="amixT")
            nc.vector.tensor_copy(amixT[:], amix_ps[:, 0:NCH * R])

            def mm1(sc):
                zt_ps = pszp.tile([R, 512], F32, tag=f"zt{sc % 2}",
                                  name=f"zt_ps{sc}")
                for gi in range(NG):
                    for g in range(NG):
                        c = gi * NG + g
                        nc.tensor.matmul(
                            zt_ps[:, 0:W], amixT[:, c * R:(c + 1) * R],
                            waves[sc][gi][:, g * W:(g + 1) * W],
                            start=(c == 0), stop=(c == NCH - 1))
                return zt_ps

            def mm2_wave(sc, zts2):
                dsb = dp.tile([128, NT, DOUT], BF16, tag="d",
                              name=f"dsb{sc}")
                for t in range(NT):
                    for ocp in (0, 2):
                        dps2 = psdp.tile([128, 1024], F32, tag="dps",
                                         name=f"dps{sc}_{t}_{ocp}")
                        tsl = slice(t * 128, (t + 1) * 128)
                        nc.tensor.matmul(
                            dps2[:, 0:512], zts2[0:R, tsl],
                            bmix2[0:R, ocp * 512:(ocp + 1) * 512],
                            tile_position=(0, 0))
                        nc.tensor.matmul(
                            dps2[:, 512:1024], zts2[32:32 + R, tsl],
                            bmix2[32:32 + R,
                                  (ocp + 1) * 512:(ocp + 2) * 512],
                            tile_position=(32, 0))
                        osl = slice(ocp * 512, (ocp + 2) * 512)
                        if ocp == 0:
                            nc.vector.tensor_copy(dsb[:, t, osl], dps2[:])
                        else:
                            nc.scalar.copy(dsb[:, t, osl], dps2[:])
                if sc + 1 < NW:
                    # one 2MB store for the whole wave
                    nc.scalar.dma_start(
                        out_d[sc * W:(sc + 1) * W, :].rearrange(
                            "(t p) o -> p t o", t=NT), dsb[:])
                else:
                    # last wave: store per s-tile to shorten the tail
                    for t in range(NT):
                        st = sc * NT + t
                        nc.scalar.dma_start(
                            out_d[st * 128:(st + 1) * 128, :],
                            dsb[:, t, :])

            # wave-0 mm1 next on the PE (amixT + wave-0 data gate it)
            zt_cur = mm1(0)

            # ---- B_mixedT replicated at strips 0-15 / 32-47 ----
            bmix2 = constp.tile([48, DOUT], BF16, tag="bmix2")
            for c4 in range(DOUT // 512):
                sl = slice(c4 * 512, (c4 + 1) * 512)
                pmix = psdp.tile([R, 512], F32, tag="dps", name=f"pmix{c4}")
                nc.tensor.matmul(pmix[:], m_sb[0], b_sb[0][:, sl],
                                 start=True, stop=False)
                nc.tensor.matmul(pmix[:], m_sb[1], b_sb[1][:, sl],
                                 start=False, stop=True)
                if c4 % 2 == 0:
                    nc.vector.tensor_copy(bmix2[0:R, sl], pmix[:])
                    nc.vector.tensor_copy(bmix2[32:32 + R, sl], pmix[:])
                else:
                    nc.scalar.copy(bmix2[0:R, sl], pmix[:])
                    nc.scalar.copy(bmix2[32:32 + R, sl], pmix[:])

            for sc in range(NW):
                zts2 = ztsp.tile([48, W], BF16, tag=f"z{sc % 2}",
                                 name=f"zts2_{sc}")
                nc.vector.tensor_copy(zts2[0:R, :], zt_cur[:, 0:W])
                nc.scalar.copy(zts2[32:32 + R, :], zt_cur[:, 0:W])
                mm2_wave(sc, zts2)
                if sc + 1 < NW:
                    zt_cur = mm1(sc + 1)

    nc.compile()
    return nc


def _in_maps(h, alpha, A_bank, B_bank):
    a_flat = A_bank.reshape(KR, DIN).astype(np.float32)
    bt_flat = B_bank.transpose(0, 2, 1).reshape(KR, DOUT).astype(np.float32)
    eye = np.eye(R, dtype=np.float32)
    maps = []
    for b in range(B):
        mix = np.kron(alpha[b].astype(np.float32).reshape(K, 1), eye)
        banks = np.concatenate([mix, a_flat, bt_flat], axis=1)
        banks = banks.reshape(2, 128, BANKW).transpose(1, 0, 2).reshape(
            128, 2 * BANKW)
        hT = np.asarray(h[b]).T  # [DIN, S]
        # wave-tile: [sc, gi, p, g*W+s] = hT[(gi*4+g)*128 + p, sc*W+s]
        hTw = hT.reshape(4, 4, 128, NW, W).transpose(3, 0, 2, 1, 4)
        hTw = hTw.reshape(NW, 4, 128, 4 * W)
        maps.append({
            "hbT": np.ascontiguousarray(hTw).astype(ml_dtypes.bfloat16),
            "banks": np.ascontiguousarray(banks.astype(ml_dtypes.bfloat16)),
        })
    return maps


def _run(inputs, trace=False):
    if "nc" not in _cache:
        _cache["nc"] = _build_nc()
    nc = _cache["nc"]
    maps = _in_maps(inputs["h"], inputs["alpha"], inputs["A_bank"],
                    inputs["B_bank"])
    res = run_bass_kernel_spmd(nc, maps, core_ids=list(range(B)), trace=trace)
    out = np.stack([res.results[b]["delta"] for b in range(B)], axis=0)
    return out.astype(np.float32), res


def kernel(**inputs):
    out, _ = _run(inputs, trace=False)
    return out


# revision 22
# speedup vs baseline: 1.0705x; 1.0434x over previous
"""LoRA basis-bank kernel for 8 TRN2 NeuronCores.

Math (per batch b):
    A_mixed  = sum_k alpha[b,k] * A_bank[k]        # [R, DIN]
    B_mixedT = sum_k alpha[b,k] * B_bank[k].T      # [R, DOUT]
    z        = h[b] @ A_mixed.T                    # [S, R]
    delta[b] = z @ B_mixedT                        # [S, DOUT]

Sharding: data-parallel over batch, 1 batch per core; banks replicated.

Host-side layout prep (no arithmetic): h shard is uploaded transposed
and wave-tiled ([wave, group, partition, 4*512] bf16) so every load is
a clean 2D DMA with 4KB contiguous rows; mix/A_bank/B_bank.T are
concatenated+folded into one [128, 2*(R+DIN+DOUT)] bf16 "banks"
tensor; alpha is expanded into a [K*R, R] block-diagonal matrix.

Device dataflow per core — single-ring wave pipeline at the memory
roofline.  The SDMA engines drain one HWDGE ring strictly in issue
order (a second ring's traffic is starved behind a queued backlog), so
loads AND stores share the sync ring with an explicitly interleaved
issue order:

    mixA | B | w0 loads | w1 loads | st0 | w2 loads | st1 |
    w3 loads | st2 | st3

Each store issue blocks the sequencer until that wave's copies land,
which is just when the ring finishes the previous queue entry — the
ring never idles and the mm1->cast->mm2->copy chain of wave k runs
entirely under wave k+1's loads / wave k-1's store.

  - A_mixT chunks land in ONE rotating PSUM buffer, one cast.
  - B_mixedT replicated at partition strips 0-15 / 32-47 (bmix2).
  - Per wave: mm1 c-outer consumes each arriving hT group tile into a
    1-bank PSUM accumulator zT [16, 512]; cast to both strips; mm2
    runs 2-way row-tiled (tile_position (0,0)/(32,0)) with each
    concurrent matmul pair writing one 2-bank [128, 1024] PSUM tile
    drained by a single wide copy (vector/scalar alternate) into a
    wave-stacked [128, 4, 2048] tile; ONE 2MB store per wave.
  - delta written bf16, upcast to fp32 on host.
"""

import ml_dtypes
import numpy as np

import concourse.bacc as bacc
import concourse.bass as bass
import concourse.mybir as mybir
import concourse.tile as tile
from concourse.bass_utils import run_bass_kernel_spmd

B, S, K, R, DIN, DOUT = 8, 2048, 16, 16, 2048, 2048
KR = K * R  # 256
BANKW = R + DIN + DOUT  # 4112
MA = R + DIN  # 2064: mix + A columns
W = 512  # wave width (s-columns per wave)
NW = S // W  # 4 waves
F32 = mybir.dt.float32
BF16 = mybir.dt.bfloat16

_cache = {}


def _build_nc():
    nc = bacc.Bacc("TRN2", target_bir_lowering=False)

    # h uploaded wave-tiled: [wave, group, partition, 4*W]
    ht_d = nc.dram_tensor("hbT", [NW, 4, 128, 4 * W], BF16,
                          kind="ExternalInput")
    bank_d = nc.dram_tensor("banks", [128, 2 * BANKW], BF16,
                            kind="ExternalInput")
    out_d = nc.dram_tensor("delta", [S, DOUT], BF16, kind="ExternalOutput")

    NCH = DIN // 128  # 16 chunks along DIN
    NG = 4            # hT group tiles per wave (4 chunks each)
    NT = W // 128     # s-tiles per wave
    with tile.TileContext(nc) as tc:
        with (
            tc.tile_pool(name="const", bufs=1) as constp,
            tc.tile_pool(name="hT", bufs=2) as hTp,
            tc.tile_pool(name="zts", bufs=1) as ztsp,
            tc.tile_pool(name="dout", bufs=2) as dp,
            tc.tile_pool(name="psz", bufs=1, space="PSUM") as pszp,
            tc.tile_pool(name="psd", bufs=3, space="PSUM") as psdp,
        ):
            bank = constp.tile([128, 2 * BANKW], BF16, tag="bank")
            # mix + A first (amix gates wave-0's mm1), then B
            nc.sync.dma_start(bank[:, 0:MA], bank_d[:, 0:MA])
            nc.sync.dma_start(bank[:, BANKW:BANKW + MA],
                              bank_d[:, BANKW:BANKW + MA])
            nc.sync.dma_start(bank[:, MA:BANKW], bank_d[:, MA:BANKW])
            nc.sync.dma_start(bank[:, BANKW + MA:2 * BANKW],
                              bank_d[:, BANKW + MA:2 * BANKW])

            def load_wave(sc):
                tiles = []
                for gi in range(NG):
                    hT = hTp.tile([128, NG * W], BF16, tag=f"g{gi}",
                                  name=f"hT_{sc}_{gi}")
                    nc.sync.dma_start(hT[:], ht_d[sc, gi])
                    tiles.append(hT)
                return tiles

            waves = [load_wave(0), load_wave(1)]

            m_sb = [bank[:, 0:R], bank[:, BANKW:BANKW + R]]
            a_sb = [bank[:, R:MA], bank[:, BANKW + R:BANKW + MA]]
            b_sb = [bank[:, MA:BANKW], bank[:, BANKW + MA:2 * BANKW]]

            # ---- A_mixT: all 16 chunks into one PSUM buffer ----
            amix_ps = psdp.tile([128, 1024], F32, tag="dps", name="amix_ps")
            for c in range(NCH):
                csl = slice(c * 128, (c + 1) * 128)
                osl = slice(c * R, (c + 1) * R)
                nc.tensor.matmul(amix_ps[:, osl], a_sb[0][:, csl], m_sb[0],
                                 start=True, stop=False)
                nc.tensor.matmul(amix_ps[:, osl], a_sb[1][:, csl], m_sb[1],
                                 start=False, stop=True)
            amixT = constp.tile([128, NCH * R], BF16, tag="amixT")
            nc.vector.tensor_copy(amixT[:], amix_ps[:, 0:NCH * R])

            # ---- B_mixedT replicated at strips 0-15 / 32-47 ----
            bmix2 = constp.tile([48, DOUT], BF16, tag="bmix2")
            for c4 in range(DOUT // 512):
                sl = slice(c4 * 512, (c4 + 1) * 512)
                pmix = psdp.tile([R, 512], F32, tag="dps", name=f"pmix{c4}")
                nc.tensor.matmul(pmix[:], m_sb[0], b_sb[0][:, sl],
                                 start=True, stop=False)
                nc.tensor.matmul(pmix[:], m_sb[1], b_sb[1][:, sl],
                                 start=False, stop=True)
                if c4 % 2 == 0:
                    nc.vector.tensor_copy(bmix2[0:R, sl], pmix[:])
                    nc.vector.tensor_copy(bmix2[32:32 + R, sl], pmix[:])
                else:
                    nc.scalar.copy(bmix2[0:R, sl], pmix[:])
                    nc.scalar.copy(bmix2[32:32 + R, sl], pmix[:])

            def mm1(sc):
                zt_ps = pszp.tile([R, 512], F32, tag=f"zt{sc % 2}",
                                  name=f"zt_ps{sc}")
                for gi in range(NG):
                    for g in range(NG):
                        c = gi * NG + g
                        nc.tensor.matmul(
                            zt_ps[:, 0:W], amixT[:, c * R:(c + 1) * R],
                            waves[sc][gi][:, g * W:(g + 1) * W],
                            start=(c == 0), stop=(c == NCH - 1))
                return zt_ps

            def mm2_wave(sc, zts2):
                dsb = dp.tile([128, NT, DOUT], BF16, tag="d",
                              name=f"dsb{sc}")
                for t in range(NT):
                    for ocp in (0, 2):
                        dps2 = psdp.tile([128, 1024], F32, tag="dps",
                                         name=f"dps{sc}_{t}_{ocp}")
                        tsl = slice(t * 128, (t + 1) * 128)
                        nc.tensor.matmul(
                            dps2[:, 0:512], zts2[0:R, tsl],
                            bmix2[0:R, ocp * 512:(ocp + 1) * 512],
                            tile_position=(0, 0))
                        nc.tensor.matmul(
                            dps2[:, 512:1024], zts2[32:32 + R, tsl],
                            bmix2[32:32 + R,
                                  (ocp + 1) * 512:(ocp + 2) * 512],
                            tile_position=(32, 0))
                        osl = slice(ocp * 512, (ocp + 2) * 512)
                        if ocp == 0:
                            nc.vector.tensor_copy(dsb[:, t, osl], dps2[:])
                        else:
                            nc.scalar.copy(dsb[:, t, osl], dps2[:])
                # one 2MB store per wave, on the SAME ring as the loads;
                # issue order interleaves it between wave loads
                nc.sync.dma_start(
                    out_d[sc * W:(sc + 1) * W, :].rearrange(
                        "(t p) o -> p t o", t=NT), dsb[:])

            zt_cur = mm1(0)
            for sc in range(NW):
                zts2 = ztsp.tile([48, W], BF16, tag=f"z{sc % 2}",
                                 name=f"zts2_{sc}")
                nc.vector.tensor_copy(zts2[0:R, :], zt_cur[:, 0:W])
                nc.scalar.copy(zts2[32:32 + R, :], zt_cur[:, 0:W])
                mm2_wave(sc, zts2)          # ...issues store(sc)
                if sc + 2 < NW:
                    waves.append(load_wave(sc + 2))  # after store(sc)
                if sc + 1 < NW:
                    zt_cur = mm1(sc + 1)

    nc.compile()
    return nc


def _in_maps(h, alpha, A_bank, B_bank):
    a_flat = A_bank.reshape(KR, DIN).astype(np.float32)
    bt_flat = B_bank.transpose(0, 2, 1).reshape(KR, DOUT).astype(np.float32)
    eye = np.eye(R, dtype=np.float32)
    maps = []
    for b in range(B):
        mix = np.kron(alpha[b].astype(np.float32).reshape(K, 1), eye)
        banks = np.concatenate([mix, a_flat, bt_flat], axis=1)
        banks = banks.reshape(2, 128, BANKW).transpose(1, 0, 2).reshape(
            128, 2 * BANKW)
        hT = np.asarray(h[b]).T  # [DIN, S]
        # wave-tile: [sc, gi, p, g*W+s] = hT[(gi*4+g)*128 + p, sc*W+s]
        hTw = hT.reshape(4, 4, 128, NW, W).transpose(3, 0, 2, 1, 4)
        hTw = hTw.reshape(NW, 4, 128, 4 * W)
        maps.append({
            "hbT": np.ascontiguousarray(hTw).astype(ml_dtypes.bfloat16),
            "banks": np.ascontiguousarray(banks.astype(ml_dtypes.bfloat16)),
        })
    return maps


def _run(inputs, trace=False):
    if "nc" not in _cache:
        _cache["nc"] = _build_nc()
    nc = _cache["nc"]
    maps = _in_maps(inputs["h"], inputs["alpha"], inputs["A_bank"],
                    inputs["B_bank"])
    res = run_bass_kernel_spmd(nc, maps, core_ids=list(range(B)), trace=trace)
    out = np.stack([res.results[b]["delta"] for b in range(B)], axis=0)
    return out.astype(np.float32), res


def kernel(**inputs):
    out, _ = _run(inputs, trace=False)
    return out


# revision 25
# speedup vs baseline: 1.1108x; 1.0376x over previous
"""LoRA basis-bank kernel for 8 TRN2 NeuronCores.

Math (per batch b):
    A_mixed  = sum_k alpha[b,k] * A_bank[k]        # [R, DIN]
    B_mixedT = sum_k alpha[b,k] * B_bank[k].T      # [R, DOUT]
    z        = h[b] @ A_mixed.T                    # [S, R]
    delta[b] = z @ B_mixedT                        # [S, DOUT]

Sharding: data-parallel over batch, 1 batch per core; banks replicated.

Host-side layout prep (no arithmetic): h shard is uploaded transposed
and wave-tiled ([wave, group, partition, 4*512] bf16) so every load is
a clean 2D DMA with 4KB contiguous rows; mix/A_bank/B_bank.T are
concatenated+folded into one [128, 2*(R+DIN+DOUT)] bf16 "banks"
tensor; alpha is expanded into a [K*R, R] block-diagonal matrix.

Device dataflow per core — single-ring wave pipeline at the memory
roofline.  The SDMA engines drain one HWDGE ring strictly in issue
order (a second ring's traffic is starved behind a queued backlog), so
loads AND stores share the sync ring with an explicitly interleaved
issue order:

    mixA | B | w0 loads | w1 loads | st0 | w2 loads | st1 |
    w3 loads | st2 | st3

Each store issue blocks the sequencer until that wave's copies land,
which is just when the ring finishes the previous queue entry — the
ring never idles and the mm1->cast->mm2->copy chain of wave k runs
entirely under wave k+1's loads / wave k-1's store.

  - A_mixT chunks land in ONE rotating PSUM buffer, one cast.
  - B_mixedT replicated at partition strips 0-15 / 32-47 (bmix2).
  - Per wave: mm1 c-outer consumes each arriving hT group tile into a
    1-bank PSUM accumulator zT [16, 512]; cast to both strips; mm2
    runs 2-way row-tiled (tile_position (0,0)/(32,0)) with each
    concurrent matmul pair writing one 2-bank [128, 1024] PSUM tile
    drained by a single wide copy (vector/scalar alternate) into a
    wave-stacked [128, 4, 2048] tile; ONE 2MB store per wave.
  - delta written bf16, upcast to fp32 on host.
"""

import ml_dtypes
import numpy as np

import concourse.bacc as bacc
import concourse.bass as bass
import concourse.mybir as mybir
import concourse.tile as tile
from concourse.bass_utils import run_bass_kernel_spmd

B, S, K, R, DIN, DOUT = 8, 2048, 16, 16, 2048, 2048
KR = K * R  # 256
BANKW = R + DIN + DOUT  # 4112
MA = R + DIN  # 2064: mix + A columns
W = 512  # wave width (s-columns per wave)
NW = S // W  # 4 waves
F32 = mybir.dt.float32
BF16 = mybir.dt.bfloat16

_cache = {}


def _build_nc():
    nc = bacc.Bacc("TRN2", target_bir_lowering=False)

    # h uploaded wave-tiled: [wave, group, partition, 4*W]
    ht_d = nc.dram_tensor("hbT", [NW, 4, 128, 4 * W], BF16,
                          kind="ExternalInput")
    bank_d = nc.dram_tensor("banks", [128, 2 * BANKW], BF16,
                            kind="ExternalInput")
    out_d = nc.dram_tensor("delta", [S, DOUT], BF16, kind="ExternalOutput")

    NCH = DIN // 128  # 16 chunks along DIN
    NG = 4            # hT group tiles per wave (4 chunks each)
    NT = W // 128     # s-tiles per wave
    with tile.TileContext(nc) as tc:
        with (
            tc.tile_pool(name="const", bufs=1) as constp,
            tc.tile_pool(name="hT", bufs=2) as hTp,
            tc.tile_pool(name="zts", bufs=1) as ztsp,
            tc.tile_pool(name="dout", bufs=2) as dp,
            tc.tile_pool(name="psz", bufs=1, space="PSUM") as pszp,
            tc.tile_pool(name="psd", bufs=3, space="PSUM") as psdp,
        ):
            bank = constp.tile([128, 2 * BANKW], BF16, tag="bank")
            # mix + A first (amix gates wave-0's mm1), then B
            nc.sync.dma_start(bank[:, 0:MA], bank_d[:, 0:MA])
            nc.sync.dma_start(bank[:, BANKW:BANKW + MA],
                              bank_d[:, BANKW:BANKW + MA])
            nc.sync.dma_start(bank[:, MA:BANKW], bank_d[:, MA:BANKW])
            nc.sync.dma_start(bank[:, BANKW + MA:2 * BANKW],
                              bank_d[:, BANKW + MA:2 * BANKW])

            def load_wave(sc):
                tiles = []
                for gi in range(NG):
                    hT = hTp.tile([128, NG * W], BF16, tag=f"g{gi}",
                                  name=f"hT_{sc}_{gi}")
                    nc.sync.dma_start(hT[:], ht_d[sc, gi])
                    tiles.append(hT)
                return tiles

            waves = [load_wave(0), load_wave(1)]

            m_sb = [bank[:, 0:R], bank[:, BANKW:BANKW + R]]
            a_sb = [bank[:, R:MA], bank[:, BANKW + R:BANKW + MA]]
            b_sb = [bank[:, MA:BANKW], bank[:, BANKW + MA:2 * BANKW]]

            # ---- A_mixT: all 16 chunks into one PSUM buffer ----
            amix_ps = psdp.tile([128, 1024], F32, tag="dps", name="amix_ps")
            for c in range(NCH):
                csl = slice(c * 128, (c + 1) * 128)
                osl = slice(c * R, (c + 1) * R)
                nc.tensor.matmul(amix_ps[:, osl], a_sb[0][:, csl], m_sb[0],
                                 start=True, stop=False)
                nc.tensor.matmul(amix_ps[:, osl], a_sb[1][:, csl], m_sb[1],
                                 start=False, stop=True)
            amixT = constp.tile([128, NCH * R], BF16, tag="amixT")
            nc.vector.tensor_copy(amixT[:], amix_ps[:, 0:NCH * R])

            # ---- PE warm-up: ~4.5us of back-to-back matmuls into a
            # scratch PSUM bank while the first wave is still loading.
            # Forces the HAM clock gate to 8/8 before the steady-state
            # wave pipeline starts; its scattered sub-us gaps then never
            # re-throttle it (re-throttle needs a fully idle ~3.4us).
            warm_ps = pszp.tile([R, 512], F32, tag="zt0", name="warm_ps")
            for i in range(14):
                nc.tensor.matmul(warm_ps[:], m_sb[0],
                                 a_sb[0][:, 0:512],
                                 start=True, stop=True)

            # ---- B_mixedT replicated at strips 0-15 / 32-47 ----
            bmix2 = constp.tile([48, DOUT], BF16, tag="bmix2")
            for c4 in range(DOUT // 512):
                sl = slice(c4 * 512, (c4 + 1) * 512)
                pmix = psdp.tile([R, 512], F32, tag="dps", name=f"pmix{c4}")
                nc.tensor.matmul(pmix[:], m_sb[0], b_sb[0][:, sl],
                                 start=True, stop=False)
                nc.tensor.matmul(pmix[:], m_sb[1], b_sb[1][:, sl],
                                 start=False, stop=True)
                if c4 % 2 == 0:
                    nc.vector.tensor_copy(bmix2[0:R, sl], pmix[:])
                    nc.vector.tensor_copy(bmix2[32:32 + R, sl], pmix[:])
                else:
                    nc.scalar.copy(bmix2[0:R, sl], pmix[:])
                    nc.scalar.copy(bmix2[32:32 + R, sl], pmix[:])

            def mm1(sc):
                zt_ps = pszp.tile([R, 512], F32, tag=f"zt{sc % 2}",
                                  name=f"zt_ps{sc}")
                for gi in range(NG):
                    for g in range(NG):
                        c = gi * NG + g
                        nc.tensor.matmul(
                            zt_ps[:, 0:W], amixT[:, c * R:(c + 1) * R],
                            waves[sc][gi][:, g * W:(g + 1) * W],
                            start=(c == 0), stop=(c == NCH - 1))
                return zt_ps

            def mm2_wave(sc, zts2):
                dsb = dp.tile([128, NT, DOUT], BF16, tag="d",
                              name=f"dsb{sc}")
                for t in range(NT):
                    for ocp in (0, 2):
                        dps2 = psdp.tile([128, 1024], F32, tag="dps",
                                         name=f"dps{sc}_{t}_{ocp}")
                        tsl = slice(t * 128, (t + 1) * 128)
                        nc.tensor.matmul(
                            dps2[:, 0:512], zts2[0:R, tsl],
                            bmix2[0:R, ocp * 512:(ocp + 1) * 512],
                            tile_position=(0, 0))
                        nc.tensor.matmul(
                            dps2[:, 512:1024], zts2[32:32 + R, tsl],
                            bmix2[32:32 + R,
                                  (ocp + 1) * 512:(ocp + 2) * 512],
                            tile_position=(32, 0))
                        osl = slice(ocp * 512, (ocp + 2) * 512)
                        if ocp == 0:
                            nc.vector.tensor_copy(dsb[:, t, osl], dps2[:])
                        else:
                            nc.scalar.copy(dsb[:, t, osl], dps2[:])
                if sc + 1 < NW:
                    # one 2MB store per wave, on the SAME ring as the
                    # loads; issue order interleaves it between waves
                    nc.sync.dma_start(
                        out_d[sc * W:(sc + 1) * W, :].rearrange(
                            "(t p) o -> p t o", t=NT), dsb[:])
                else:
                    # last wave: store per s-tile so the tail pipelines
                    for t in range(NT):
                        st = sc * NT + t
                        nc.sync.dma_start(
                            out_d[st * 128:(st + 1) * 128, :],
                            dsb[:, t, :])

            zt_cur = mm1(0)
            for sc in range(NW):
                zts2 = ztsp.tile([48, W], BF16, tag=f"z{sc % 2}",
                                 name=f"zts2_{sc}")
                nc.vector.tensor_copy(zts2[0:R, :], zt_cur[:, 0:W])
                nc.scalar.copy(zts2[32:32 + R, :], zt_cur[:, 0:W])
                mm2_wave(sc, zts2)          # ...issues store(sc)
                if sc + 2 < NW:
                    waves.append(load_wave(sc + 2))  # after store(sc)
                if sc + 1 < NW:
                    zt_cur = mm1(sc + 1)

    nc.compile()
    return nc


def _in_maps(h, alpha, A_bank, B_bank):
    a_flat = A_bank.reshape(KR, DIN).astype(np.float32)
    bt_flat = B_bank.transpose(0, 2, 1).reshape(KR, DOUT).astype(np.float32)
    eye = np.eye(R, dtype=np.float32)
    maps = []
    for b in range(B):
        mix = np.kron(alpha[b].astype(np.float32).reshape(K, 1), eye)
        banks = np.concatenate([mix, a_flat, bt_flat], axis=1)
        banks = banks.reshape(2, 128, BANKW).transpose(1, 0, 2).reshape(
            128, 2 * BANKW)
        hT = np.asarray(h[b]).T  # [DIN, S]
        # wave-tile: [sc, gi, p, g*W+s] = hT[(gi*4+g)*128 + p, sc*W+s]
        hTw = hT.reshape(4, 4, 128, NW, W).transpose(3, 0, 2, 1, 4)
        hTw = hTw.reshape(NW, 4, 128, 4 * W)
        maps.append({
            "hbT": np.ascontiguousarray(hTw).astype(ml_dtypes.bfloat16),
            "banks": np.ascontiguousarray(banks.astype(ml_dtypes.bfloat16)),
        })
    return maps


def _run(inputs, trace=False):
    if "nc" not in _cache:
        _cache["nc"] = _build_nc()
    nc = _cache["nc"]
    maps = _in_maps(inputs["h"], inputs["alpha"], inputs["A_bank"],
                    inputs["B_bank"])
    res = run_bass_kernel_spmd(nc, maps, core_ids=list(range(B)), trace=trace)
    out = np.stack([res.results[b]["delta"] for b in range(B)], axis=0)
    return out.astype(np.float32), res


def kernel(**inputs):
    out, _ = _run(inputs, trace=False)
    return out
